# revision 25
# baseline (speedup 1.0000x reference)
"""Cumulative LayerNorm Trainium2 Bass kernel (v2: single-stats, bf16 path).

x: [B=8, C=256, T=16000] f32.  Per timestep t: normalize x[:, :, t] by the
mean/std of all elements x[:, :, t'<=t] (cumulative over channels+time), then
scale by weight[c] and add bias[c].

Sharding: pure data parallel over B across 8 NeuronCores (1 sample/core).

Per-core plan (C=256 = 2 halves of 128 partitions, T on the free dim).
All 8 io-tiles of x stay resident in SBUF as bf16 (64 KB/partition), so the
kernel runs as two DMA-bound epochs with one tiny serial stats step between:

  Phase A (x8 io-tiles of 2000 cols):
    - gpsimd cast-DMA loads x f32 HBM -> bf16 SBUF (one DMA per tile).
    - xx = x^2 (DVE, bf16 2x mode, one FD-4000 op per tile).
    - PE: per 500-col block one accumulation group of 4 bf16 matmuls with
      half-zero weights ([1100] / [0011] columns) sums s over both channel
      halves into PSUM rows 0-1 and sq into rows 2-3.
    - ACT evacuates [4, 1000] PSUM -> bf16, one DMA scatters it into the
      [128, 250] stat surface (t = 125*p + i; s in cols 0:125, sq 125:250).
  Stats (once):
    - DVE tensor_tensor_scan over the full [128, 125] stat rows (fp32
      accumulator over bf16 terms), strict-upper-triangular f32r matmul for
      exclusive cross-partition offsets, then mean / var / istd / -mean.
  Phase C (x8 io-tiles, per 1000-col half):
    - one [1, 2000] gather DMA per field per tile back into row layout,
      PE rank-1 broadcasts (ones x row) into PSUM, ACT evacuates to bf16.
    - z = x + (-mean) on GPSIMD; y = (z * w[p]) * istd on DVE (bf16 2x).
    - gpsimd cast-DMA store bf16 SBUF -> f32 HBM (one DMA per tile).
"""
import ml_dtypes
import numpy as np

B, C, T = 8, 256, 16000
P = 128
NH = 2                     # channel halves
CHUNK = 2000               # t per io-tile
NCHUNK = T // CHUNK        # 8
ROWS = T // P              # 125  (stat layout free dim; t = 125*p + i)
PB = 500                   # psum block columns
EPS = 1e-06

_cached = {}


def _build_nc(with_bias: bool):
    from contextlib import ExitStack

    import concourse.tile as tile
    from concourse import bacc, mybir

    f32 = mybir.dt.float32
    f32r = mybir.dt.float32r
    bf16 = mybir.dt.bfloat16
    ALU = mybir.AluOpType
    ACTF = mybir.ActivationFunctionType

    nc = bacc.Bacc()

    x = nc.dram_tensor("x", [C, T], f32, kind="ExternalInput")
    wvec = nc.dram_tensor("wvec", [C, 1], f32, kind="ExternalInput")
    tri_d = nc.dram_tensor("tri", [P, P], f32r, kind="ExternalInput")
    oz2_d = nc.dram_tensor("oz2", [P, 2], bf16, kind="ExternalInput")
    zo2_d = nc.dram_tensor("zo2", [P, 2], bf16, kind="ExternalInput")
    onesb_d = nc.dram_tensor("onesb", [1, P], f32r, kind="ExternalInput")
    invcnt_d = nc.dram_tensor("invcnt", [P, ROWS], f32, kind="ExternalInput")
    if with_bias:
        bvec = nc.dram_tensor("bvec", [C, 1], f32, kind="ExternalInput")
    y = nc.dram_tensor("y", [C, T], f32, kind="ExternalOutput")

    # DRAM views with channel halves split onto the partition dim.
    x_v = x.rearrange("(h p) t -> p h t", h=NH)
    y_v = y.rearrange("(h p) t -> p h t", h=NH)

    with tile.TileContext(nc) as tc, ExitStack() as ctx:
        const = ctx.enter_context(tc.tile_pool(name="const", bufs=1))
        persist = ctx.enter_context(tc.tile_pool(name="persist", bufs=1))
        sqpool = ctx.enter_context(tc.tile_pool(name="sqpool", bufs=2))
        rowpool = ctx.enter_context(tc.tile_pool(name="rowpool", bufs=3))
        browp = ctx.enter_context(tc.tile_pool(name="browp", bufs=4))
        bcpool = ctx.enter_context(tc.tile_pool(name="bcpool", bufs=4))
        zpool = ctx.enter_context(tc.tile_pool(name="zpool", bufs=4))
        ypool = ctx.enter_context(tc.tile_pool(name="ypool", bufs=2))
        ps_stat = ctx.enter_context(tc.tile_pool(name="ps_stat", bufs=2, space="PSUM"))
        ps_nm = ctx.enter_context(tc.tile_pool(name="ps_nm", bufs=1, space="PSUM"))
        ps_i = ctx.enter_context(tc.tile_pool(name="ps_i", bufs=1, space="PSUM"))

        # ---- constants ----
        tri = const.tile([P, P], f32r)
        nc.sync.dma_start(out=tri, in_=tri_d[:, :])
        oz2 = const.tile([P, 2], bf16)
        nc.sync.dma_start(out=oz2, in_=oz2_d[:, :])
        zo2 = const.tile([P, 2], bf16)
        nc.sync.dma_start(out=zo2, in_=zo2_d[:, :])
        onesb = const.tile([1, P], f32r)
        nc.sync.dma_start(out=onesb, in_=onesb_d[:, :])
        invcnt = const.tile([P, ROWS], f32)
        nc.sync.dma_start(out=invcnt, in_=invcnt_d[:, :])
        w_sb = const.tile([P, NH], f32)
        for h in range(NH):
            nc.sync.dma_start(out=w_sb[:, h : h + 1], in_=wvec[h * P : (h + 1) * P, 0:1])
        if with_bias:
            b_sb = const.tile([P, NH], f32)
            for h in range(NH):
                nc.sync.dma_start(
                    out=b_sb[:, h : h + 1], in_=bvec[h * P : (h + 1) * P, 0:1]
                )
        eps_sb = const.tile([P, 1], f32)
        nc.vector.memset(eps_sb, EPS)

        # ---- persistent stat surfaces ----
        scombo = persist.tile([P, 2, ROWS], bf16)   # [:, 0, :]=s  [:, 1, :]=sq
        s_cs = persist.tile([P, ROWS], f32)         # prefix sums (within-row)
        sq_cs = persist.tile([P, ROWS], f32)
        mean_t = persist.tile([P, ROWS], f32)
        ex2_t = persist.tile([P, ROWS], f32)        # E[x^2] -> var
        msq_t = persist.tile([P, ROWS], f32)        # mean^2 -> sqrt(var+eps)
        istd_t = persist.tile([P, ROWS], f32)
        nm_t = persist.tile([P, ROWS], f32)         # -mean
        st_sb = persist.tile([P, 2], f32r)          # per-row totals (s, sq)
        # Zeroed so the half-0 triangular matmul reads 0 (not garbage) in the
        # not-yet-written rows 64..127 (tri zeros them, but 0*NaN = NaN).
        nc.vector.memset(st_sb.bitcast(f32), 0.0)

        # All of x stays resident in SBUF as bf16 (64 KB/partition); loaded
        # by 8 per-tile cast-DMAs (2 MB f32 reads each), all issued up front.
        xfull = persist.tile([P, NH, T], bf16)

        def load_tile(tix):
            qs = slice(tix * CHUNK, (tix + 1) * CHUNK)
            nc.gpsimd.dma_start(out=xfull[:, :, qs], in_=x_v[:, :, qs])

        def phase_a(tix):
            t0 = tix * CHUNK
            xb = xfull[:, :, t0 : t0 + CHUNK]
            xx = sqpool.tile([P, NH, CHUNK], bf16, tag="xx", name="xx")
            nc.vector.tensor_tensor(xx, xb, xb, ALU.mult)
            for a2 in range(2):  # 1000-col groups
                sp = ps_stat.tile([2, 2, 512], f32, tag="stat", name="sp")
                for j in range(2):
                    cs = slice((2 * a2 + j) * PB, (2 * a2 + j + 1) * PB)
                    csl = slice(t0 + (2 * a2 + j) * PB,
                                t0 + (2 * a2 + j + 1) * PB)
                    # row 0 accumulates s (oz2 = [1,0] columns),
                    # row 1 accumulates sq (zo2 = [0,1]).
                    nc.tensor.matmul(sp[:, j, 0:PB], oz2, xfull[:, 0, csl],
                                     start=True, stop=False)
                    nc.tensor.matmul(sp[:, j, 0:PB], oz2, xfull[:, 1, csl],
                                     start=False, stop=False)
                    nc.tensor.matmul(sp[:, j, 0:PB], zo2, xx[:, 0, cs],
                                     start=False, stop=False)
                    nc.tensor.matmul(sp[:, j, 0:PB], zo2, xx[:, 1, cs],
                                     start=False, stop=True)
                rowt = rowpool.tile([2, 2, PB], bf16, tag="rowt", name="rowt")
                nc.scalar.copy(rowt, sp[:, :, 0:PB])
                # rows 16*tix+8*a2 .. +8 of the stat layout, s then sq.
                rp = 16 * tix + 8 * a2
                nc.sync.dma_start(
                    out=scombo[rp : rp + 8, 0, :], in_=rowt[0:1, :, :]
                )
                nc.sync.dma_start(
                    out=scombo[rp : rp + 8, 1, :], in_=rowt[1:2, :, :]
                )

        def stats(hf):
            """Prefix stats for stat-layout rows 32*hf .. 32*hf+32 (io-tiles
            2*hf, 2*hf+1).  The tri matmul contracts all 128 st_sb rows;
            future rows are zero so the exclusive offsets stay exact."""
            sl = slice(32 * hf, 32 * hf + 32)
            sv = scombo[sl, 0, :]
            qv = scombo[sl, 1, :]
            nc.vector.tensor_tensor_scan(
                out=s_cs[sl, :], data0=sv, data1=sv, initial=0.0,
                op0=ALU.add, op1=ALU.bypass,
            )
            nc.vector.tensor_tensor_scan(
                out=sq_cs[sl, :], data0=qv, data1=qv, initial=0.0,
                op0=ALU.add, op1=ALU.bypass,
            )
            nc.vector.tensor_copy(st_sb[sl, 0:1], s_cs[sl, ROWS - 1 : ROWS])
            nc.vector.tensor_copy(st_sb[sl, 1:2], sq_cs[sl, ROWS - 1 : ROWS])
            offps = ps_stat.tile([P, 2], f32, tag="stat", name="offps")
            nc.tensor.matmul(offps, tri, st_sb, start=True, stop=True)
            nc.vector.scalar_tensor_tensor(
                out=mean_t[sl, :], in0=s_cs[sl, :], scalar=offps[sl, 0:1],
                in1=invcnt[sl, :], op0=ALU.add, op1=ALU.mult,
            )
            nc.vector.scalar_tensor_tensor(
                out=ex2_t[sl, :], in0=sq_cs[sl, :], scalar=offps[sl, 1:2],
                in1=invcnt[sl, :], op0=ALU.add, op1=ALU.mult,
            )
            nc.vector.tensor_scalar_mul(nm_t[sl, :], mean_t[sl, :], -1.0)
            nc.vector.tensor_tensor(msq_t[sl, :], mean_t[sl, :], mean_t[sl, :],
                                    ALU.mult)
            nc.vector.tensor_tensor(ex2_t[sl, :], ex2_t[sl, :], msq_t[sl, :],
                                    ALU.subtract)
            nc.scalar.activation(msq_t[sl, :], ex2_t[sl, :], ACTF.Sqrt,
                                 bias=eps_sb[sl, :], scale=1.0)
            nc.vector.reciprocal(out=istd_t[sl, :], in_=msq_t[sl, :])

        def phase_c(tix):
            t0 = tix * CHUNK
            rsl = slice(16 * tix, 16 * tix + 16)
            # gathers ride the scalar HWDGE queue so they can't head-of-line
            # block the stat scatters on sync while waiting for stats(hf)
            nmrow = browp.tile([1, CHUNK], f32, tag="brow", name="nmrow")
            nc.scalar.dma_start(out=nmrow, in_=nm_t[rsl, :])
            isrow = browp.tile([1, CHUNK], f32, tag="brow", name="isrow")
            nc.scalar.dma_start(out=isrow, in_=istd_t[rsl, :])
            y_t = ypool.tile([P, NH, CHUNK], bf16, tag="y", name="y_t")
            for half in range(2):  # 1000-col halves
                nmps = ps_nm.tile([P, 2, 512], f32, tag="nm", name="nmps")
                isps = ps_i.tile([P, 2, 512], f32, tag="ibc", name="isps")
                for j in range(2):
                    cs = slice((2 * half + j) * PB, (2 * half + j + 1) * PB)
                    nc.tensor.matmul(nmps[:, j, 0:PB], onesb,
                                     nmrow[0:1, cs].bitcast(f32r),
                                     start=True, stop=True)
                    nc.tensor.matmul(isps[:, j, 0:PB], onesb,
                                     isrow[0:1, cs].bitcast(f32r),
                                     start=True, stop=True)
                nm_sb = bcpool.tile([P, 2, PB], bf16, tag="bc", name="nm_sb")
                nc.scalar.copy(nm_sb, nmps[:, :, 0:PB])
                is_sb = bcpool.tile([P, 2, PB], bf16, tag="bc", name="is_sb")
                nc.scalar.copy(is_sb, isps[:, :, 0:PB])
                ccol = slice(half * 1000, (half + 1) * 1000)
                gcol = slice(t0 + half * 1000, t0 + (half + 1) * 1000)
                for h in range(NH):
                    x_ap = xfull[:, h, gcol].rearrange("p (j n) -> p j n", j=2)
                    ys = y_t[:, h, ccol].rearrange("p (j n) -> p j n", j=2)
                    z = zpool.tile([P, 2, PB], bf16, tag="z", name="z")
                    nc.vector.tensor_tensor(z, x_ap, nm_sb, ALU.add)
                    u = zpool.tile([P, 2, PB], bf16, tag="z", name="u")
                    nc.vector.tensor_tensor(u, z, is_sb, ALU.mult)
                    # y = u * w[p]  (tensor_scalar runs 4x/cycle on bf16)
                    if with_bias:
                        nc.vector.tensor_scalar(
                            out=ys, in0=u, scalar1=w_sb[:, h : h + 1],
                            scalar2=b_sb[:, h : h + 1],
                            op0=ALU.mult, op1=ALU.add,
                        )
                    else:
                        nc.vector.tensor_scalar_mul(ys, u, w_sb[:, h : h + 1])
                nc.gpsimd.dma_start(
                    out=y_v[:, :, t0 + half * 1000 : t0 + (half + 1) * 1000],
                    in_=y_t[:, :, ccol],
                )

        # Emission: all loads lead the gpsimd queue (stores must never block
        # them).  Stats run in 32-row chunks right after each io-tile pair's
        # phase A, so the first y stores start while the loads are still
        # streaming and the HBM pipe never drains mid-kernel.
        for tix in range(NCHUNK):
            load_tile(tix)
        phase_a(0)
        phase_a(1)
        stats(0)
        phase_c(0)
        phase_a(2)
        phase_c(1)
        phase_a(3)
        stats(1)
        phase_c(2)
        phase_a(4)
        phase_c(3)
        phase_a(5)
        stats(2)
        phase_c(4)
        phase_a(6)
        phase_c(5)
        phase_a(7)
        stats(3)
        phase_c(6)
        phase_c(7)
    nc.compile()
    return nc


def _consts():
    tri = np.triu(np.ones((P, P), dtype=np.float32), k=1)  # tri[k,m]=1 iff k<m
    oz2 = np.zeros((P, 2), dtype=np.float32)
    oz2[:, 0] = 1.0
    zo2 = np.zeros((P, 2), dtype=np.float32)
    zo2[:, 1] = 1.0
    onesb = np.ones((1, P), dtype=np.float32)
    t_idx = (ROWS * np.arange(P, dtype=np.float64)[:, None]
             + np.arange(ROWS, dtype=np.float64)[None, :])
    invcnt = (1.0 / (C * (t_idx + 1.0))).astype(np.float32)
    return {"tri": tri, "oz2": oz2.astype(ml_dtypes.bfloat16),
            "zo2": zo2.astype(ml_dtypes.bfloat16), "onesb": onesb,
            "invcnt": invcnt}


def _get_nc(with_bias: bool):
    key = ("nc", with_bias)
    if key not in _cached:
        _cached[key] = _build_nc(with_bias)
    return _cached[key]


def _run(x, weight, bias, trace=False):
    from concourse.bass_utils import run_bass_kernel_spmd

    x = np.ascontiguousarray(np.asarray(x, dtype=np.float32))
    weight = np.asarray(weight, dtype=np.float32).reshape(C, 1)
    bias = np.asarray(bias, dtype=np.float32).reshape(C, 1)
    with_bias = bool(np.any(bias))
    nc = _get_nc(with_bias)

    consts = _consts()
    in_maps = []
    for b in range(B):
        m = {"x": np.ascontiguousarray(x[b]), "wvec": weight}
        if with_bias:
            m["bvec"] = bias
        m.update(consts)
        in_maps.append(m)

    res = run_bass_kernel_spmd(nc, in_maps, core_ids=list(range(B)), trace=trace)
    y = np.stack([r["y"] for r in res.results], axis=0)
    return y, res


def kernel(x, weight, bias):
    y, _ = _run(x, weight, bias, trace=False)
    return y
